# revision 18
# baseline (speedup 1.0000x reference)
"""Causal self-attention with ALiBi for Trainium2 (8 NeuronCores).

Sharding: 8 cores = 4 batch x 2 head-groups. Head-groups are interleaved
(even/odd ranks of ALiBi slope) so the exact-zero tile skipping (see below)
is load-balanced and the SPMD program is identical on every core.

Math: reference computes softmax(q.k/8 + slope*(q-k), causal) @ v.
We factor exp(q.k/8 + slope*(q-k) - slope*q - C) = exp(q.k/8) * g[k],
g[k] = exp(-slope*k - C).  g is folded into V (and into the appended
"ones" column that accumulates the softmax denominator), so the ACT
exp is a pure exp(x/8) with no per-row bias and the whole attention
runs in the transposed [k-partition, q-free] layout:

  S_T[k,q]  = k . q            (PE, fp32r, 2 heads packed via row tiling)
  P_T[k,q]  = exp(S_T/8)       (ACT, batched over 2 k-tiles)
  acc[d,q] += (v*g | g)^T P_T  (PE, M=65: 64 dims + denominator row)
  o_T[d,q]  = acc[d,q] / acc[64,q]
  y[q,e]    = o_T^T @ Wo       (PE, natural output layout)

Tiles where g underflows to exactly 0.0 in fp32 contribute exact zeros
and are skipped (host-verified per head; program uses the max keep over
the two head-groups so all 8 cores share one NEFF).
"""

import os
import sys

sys.path.insert(0, "/opt/trn_rl_repo")

import numpy as np

import concourse.bass as bass
import concourse.mybir as mybir
import concourse.tile as tile
from concourse import bacc
from concourse import bass_utils

F32 = mybir.dt.float32
F32R = mybir.dt.float32r
EXP = mybir.ActivationFunctionType.Exp

ALIBI_SLOPES = [
    0.6299605249, 0.396850263, 0.25, 0.1574901312,
    0.0992125657, 0.0625, 0.0393725328, 0.0248031414,
    0.015625, 0.0098431332, 0.0062007854, 0.00390625,
]
B, S, E, H, D = 4, 2048, 768, 12, 64
HL = 6                 # local heads per core
EC = HL * D            # 384 local embed
NJ = 4                 # q chunks of 512
QW = 512               # q chunk width
NK = 16                # k tiles of 128
NEC = 6                # E contraction tiles (768/128)
NM = 3                 # local e' tiles (384/128)
CSHIFT = 4.0           # softmax shift constant C
NEG = -1.0e30

# Slot l on group-A cores holds HEADS_A[l], on group-B cores HEADS_B[l].
# Pairing constraint: (2048*h) % 12 must match per slot (PHI) so the
# output-scramble program is identical on all cores; within that, heads
# are paired to balance the ALiBi-decay tile skipping.
HEADS_A = [0, 6, 2, 8, 4, 10]
HEADS_B = [3, 9, 5, 11, 1, 7]
PHI = [0, 0, 4, 4, 8, 8]
NC = 172               # z columns per slot (= ceil((2047+8)/12)+1)


def _g_table(heads):
    """[128, 96] fp32: g2[p, 6*jk + l] = exp(-slope_l*(128*jk+p) - C)."""
    k = np.arange(S, dtype=np.float64)
    sl = np.array([ALIBI_SLOPES[h] for h in heads], dtype=np.float64)
    g = np.exp(-np.outer(sl, k) - CSHIFT).astype(np.float32)  # [6, 2048]
    a = g.reshape(HL, NK, 128).transpose(2, 1, 0)             # [128, 16, 6]
    return np.ascontiguousarray(a.reshape(128, NK * HL)), a


def _keep_per_slot():
    """Per local-head-slot number of k-tiles with any nonzero g, maxed
    over the two head groups (so one program serves all cores)."""
    _, ga = _g_table(HEADS_A)
    _, gb = _g_table(HEADS_B)
    keep = []
    for l in range(HL):
        kp = 1
        for jk in range(NK):
            if (ga[:, jk, l] > 0).any() or (gb[:, jk, l] > 0).any():
                kp = jk + 1
        keep.append(kp)
    return keep


KEEP = _keep_per_slot()


def _masks():
    """[128, 4*512] additive causal masks for the 4 diagonal sub-positions.
    m[p, jd*512 + c] = 0 if 128*jd + p <= c else -1e30."""
    p = np.arange(128)[:, None]
    out = np.zeros((128, 4, QW), dtype=np.float32)
    for jd in range(4):
        c = np.arange(QW)[None, :]
        out[:, jd, :] = np.where(128 * jd + p <= c, 0.0, NEG)
    return np.ascontiguousarray(out.reshape(128, 4 * QW))


def _emit(tc, io):
    nc = tc.nc
    xT, wq, wk, wv, wo, bqt, bkt, bv, gt, mk, y = (
        io["xT"], io["wq"], io["wk"], io["wv"], io["wo"], io["bqt"],
        io["bkt"], io["bv"], io["gt"], io["mk"], io["y"])

    import contextlib
    ctx = tc._emit_ctx = contextlib.ExitStack()

    consts = ctx.enter_context(tc.tile_pool(name="consts", bufs=1))
    big = ctx.enter_context(tc.tile_pool(name="big", bufs=1))

    # ---- persistent SBUF ----
    wo_sb = consts.tile([128, 6, E], F32R, name="wo_sb")
    # packed small constants: [bqt | bkt | g | bv] = 3 + 3 + 96 + 384 cols
    small = consts.tile([128, 486], F32, name="small")
    bqt_sb = small[:, 0:NM]
    bkt_sb = small[:, NM:2 * NM]
    g_sb = small[:, 6:6 + NK * HL]
    bv_sb = small[:, 102:102 + EC]

    qT = big.tile([128, NM, S], F32R, name="qT")
    kT = big.tile([128, NM, S], F32R, name="kT")
    v_all = big.tile([128, NK, HL * 65], F32R, name="v_all")

    for t in range(6):
        nc.sync.dma_start(out=wo_sb[:, t, :], in_=wo[128 * t:128 * (t + 1), :])
    nc.sync.dma_start(out=g_sb, in_=gt)
    nc.sync.dma_start(out=bqt_sb, in_=bqt)
    nc.sync.dma_start(out=bkt_sb, in_=bkt)
    nc.sync.dma_start(
        out=bv_sb,
        in_=bass.AP(tensor=bv.tensor, offset=0, ap=[[0, 128], [1, EC]]))

    # ---- psum pools ----
    pp = ctx.enter_context(tc.tile_pool(name="pp", bufs=2, space="PSUM"))
    sc = ctx.enter_context(tc.tile_pool(name="sc", bufs=2, space="PSUM"))
    accp = ctx.enter_context(tc.tile_pool(name="accp", bufs=2, space="PSUM"))

    # ---- phase 1: projections (inside a scoped pool so xT/w free after) ----
    with tc.tile_pool(name="xw", bufs=1) as xw:
        xT_sb = xw.tile([128, NEC, S], F32R, name="xT_sb")
        wq_sb = xw.tile([128, NEC, EC], F32R, name="wq_sb")
        wk_sb = xw.tile([128, NEC, EC], F32R, name="wk_sb")
        wv_sb = xw.tile([128, NEC, EC], F32R, name="wv_sb")
        for c in range(NEC):
            nc.sync.dma_start(out=xT_sb[:, c, :], in_=xT[128 * c:128 * (c + 1), :])
            nc.sync.dma_start(out=wq_sb[:, c, :], in_=wq[128 * c:128 * (c + 1), :])
            nc.sync.dma_start(out=wk_sb[:, c, :], in_=wk[128 * c:128 * (c + 1), :])
            nc.sync.dma_start(out=wv_sb[:, c, :], in_=wv[128 * c:128 * (c + 1), :])

        for w_sb, b_sb, dst in ((wq_sb, bqt_sb, qT), (wk_sb, bkt_sb, kT)):
            for m in range(NM):
                for j in range(NJ):
                    ps = pp.tile([128, QW], F32, name="ps_qk", tag="pp")
                    for c in range(NEC):
                        nc.tensor.matmul(
                            ps,
                            lhsT=w_sb[:, c, 128 * m:128 * (m + 1)],
                            rhs=xT_sb[:, c, QW * j:QW * (j + 1)],
                            start=(c == 0), stop=(c == NEC - 1))
                    nc.vector.tensor_scalar_add(
                        out=dst[:, m, QW * j:QW * (j + 1)], in0=ps,
                        scalar1=b_sb[:, m:m + 1])

        for jk in range(NK):
            psv = pp.tile([128, EC], F32, name="psv", tag="pp")
            for c in range(NEC):
                nc.tensor.matmul(
                    psv,
                    lhsT=xT_sb[:, c, 128 * jk:128 * (jk + 1)],
                    rhs=wv_sb[:, c, :],
                    start=(c == 0), stop=(c == NEC - 1))
            vst = xw.tile([128, EC], F32, name="vst", tag="vst", bufs=2)
            nc.vector.tensor_add(vst, psv, bv_sb)
            for l in range(HL):
                nc.vector.tensor_scalar_mul(
                    out=v_all[:, jk, 65 * l:65 * l + 64],
                    in0=vst[:, 64 * l:64 * (l + 1)],
                    scalar1=g_sb[:, HL * jk + l:HL * jk + l + 1])
            # denominator column = g
            nc.vector.tensor_copy(
                out=v_all[:, jk, :].rearrange("p (l c) -> p l c", c=65)[:, :, 64:65],
                in_=g_sb[:, HL * jk:HL * (jk + 1)].rearrange(
                    "p (l o) -> p l o", o=1))

    # ---- pools living from phase 2 on (reuse xw's released space) ----
    att = ctx.enter_context(tc.tile_pool(name="att", bufs=1))
    oT = att.tile([128, NM, S], F32, name="oT")
    z_all = att.tile([128, HL, 6, NC], F32R, name="z_all")
    mk_sb = att.tile([128, 4, QW], F32, name="mk_sb")
    nc.sync.dma_start(out=mk_sb[:, :, :],
                      in_=mk.rearrange("p (a b) -> p a b", a=4))
    nc.gpsimd.memset(z_all.bitcast(F32), 0.0)
    pt = ctx.enter_context(tc.tile_pool(name="pt", bufs=4))
    ypool = ctx.enter_context(tc.tile_pool(name="ypool", bufs=2))
    nrm = ctx.enter_context(tc.tile_pool(name="nrm", bufs=2))

    # ---- phases 2-3: attention; pair-outer so z-copies start early ----
    for t in range(NM):              # head-slot pair (2t, 2t+1)
        for j in range(NJ):
            lims = [min(4 * j + 4, KEEP[2 * t + hh]) for hh in range(2)]
            ngroups = (max(lims) + 1) // 2
            accs = []
            for hh in range(2):
                acc = accp.tile([65, QW], F32, name="acc", tag="acc")
                accs.append(acc)
            for g in range(ngroups):
                sco = [None, None]
                jks = [None, None]
                for hh in range(2):
                    jj = [k for k in (2 * g, 2 * g + 1) if k < lims[hh]]
                    jks[hh] = jj
                    if not jj:
                        continue
                    s_ps = sc.tile([128, len(jj), QW], F32, name="s_ps", tag="sc")
                    sco[hh] = s_ps
                    for i, jk in enumerate(jj):
                        nc.tensor.matmul(
                            s_ps[:, i, :],
                            lhsT=kT[64 * hh:64 * hh + 64, t,
                                    128 * jk:128 * (jk + 1)],
                            rhs=qT[64 * hh:64 * hh + 64, t,
                                   QW * j:QW * (j + 1)],
                            start=True, stop=True,
                            tile_position=(64 * hh, 0))
                for hh in range(2):
                    if sco[hh] is None:
                        continue
                    for i, jk in enumerate(jks[hh]):
                        if jk >= 4 * j:
                            nc.vector.tensor_add(
                                sco[hh][:, i, :], sco[hh][:, i, :],
                                mk_sb[:, jk - 4 * j, :])
                    p_sb = pt.tile([128, len(jks[hh]), QW], F32R,
                                   name="p_sb", tag="pt")
                    nc.scalar.activation(out=p_sb, in_=sco[hh], func=EXP,
                                         bias=0.0, scale=0.125)
                    l = 2 * t + hh
                    for i, jk in enumerate(jks[hh]):
                        nc.tensor.matmul(
                            accs[hh],
                            lhsT=v_all[:, jk, 65 * l:65 * (l + 1)],
                            rhs=p_sb[:, i, :],
                            start=(jk == 0), stop=(jk == lims[hh] - 1))
            # normalize: o = acc[0:64] / acc[64]
            for hh in range(2):
                rr = nrm.tile([1, QW], F32, name="rr", tag="rr")
                nc.vector.reciprocal(out=rr, in_=accs[hh][64:65, :])
                bc = nrm.tile([64, QW], F32, name="bc", tag="bc")
                nc.gpsimd.partition_broadcast(bc, rr)
                nc.vector.tensor_mul(
                    oT[64 * hh:64 * hh + 64, t, QW * j:QW * (j + 1)],
                    accs[hh][0:64, :], bc)
        # scramble this pair's output into the head-major-flatten layout:
        # z[l][64j+d, c] = o_l[d, s], s = 12*(c-c0) + s0
        for hh in range(2):
            l = 2 * t + hh
            phi = PHI[l]
            for jb in range(12):
                s0 = (jb - phi) % 12
                cnt = (S - 1 - s0) // 12 + 1
                c0 = 1 if jb < phi else 0
                nc.vector.tensor_copy(
                    out=z_all[64 * (jb % 2):64 * (jb % 2) + 64, l, jb // 2,
                              c0:c0 + cnt],
                    in_=oT[64 * hh:64 * hh + 64, t,
                           s0:s0 + 12 * (cnt - 1) + 1:12])

    # ---- phase 4: out-projection per slot ----
    for l in range(HL):
        for cc, (r0, r1) in enumerate(((0, 128), (128, NC))):
            yst = ypool.tile([128, E], F32, name="yst", tag="y")
            for n in range(2):
                psy = pp.tile([128, EC], F32, name="psy", tag="pp")
                for m in range(NM * 2):
                    nc.tensor.matmul(
                        psy[0:r1 - r0, :],
                        lhsT=z_all[:, l, m, r0:r1],
                        rhs=wo_sb[:, m, EC * n:EC * (n + 1)],
                        start=(m == 0), stop=(m == NM * 2 - 1))
                nc.vector.tensor_copy(out=yst[0:r1 - r0, EC * n:EC * (n + 1)],
                                      in_=psy[0:r1 - r0, :])
            nc.sync.dma_start(out=y[l, r0:r1, :], in_=yst[0:r1 - r0, :])

    ctx.close()


_BUILT = None
LAST_RESULT = None


def _build():
    global _BUILT
    if _BUILT is not None:
        return _BUILT
    nc = bacc.Bacc("TRN2", target_bir_lowering=False, debug=False,
                   enable_asserts=False, num_devices=8)
    io = {
        "xT": nc.dram_tensor("xT", [E, S], F32R, kind="ExternalInput").ap(),
        "wq": nc.dram_tensor("wq", [E, EC], F32R, kind="ExternalInput").ap(),
        "wk": nc.dram_tensor("wk", [E, EC], F32R, kind="ExternalInput").ap(),
        "wv": nc.dram_tensor("wv", [E, EC], F32R, kind="ExternalInput").ap(),
        "wo": nc.dram_tensor("wo", [E, E], F32R, kind="ExternalInput").ap(),
        "bqt": nc.dram_tensor("bqt", [128, NM], F32, kind="ExternalInput").ap(),
        "bkt": nc.dram_tensor("bkt", [128, NM], F32, kind="ExternalInput").ap(),
        "bv": nc.dram_tensor("bv", [1, EC], F32, kind="ExternalInput").ap(),
        "gt": nc.dram_tensor("gt", [128, NK * HL], F32,
                             kind="ExternalInput").ap(),
        "mk": nc.dram_tensor("mk", [128, 4 * QW], F32,
                             kind="ExternalInput").ap(),
        "y": nc.dram_tensor("y", [HL, NC, E], F32, kind="ExternalOutput").ap(),
    }
    with tile.TileContext(nc) as tc:
        _emit(tc, io)
    nc.compile()
    _BUILT = nc
    return nc


def _core_inputs(x, Wq, bq, Wk, bk, Wv, bv, Wo, b, hg):
    heads = HEADS_A if hg == 0 else HEADS_B
    cols = np.concatenate([np.arange(h * D, (h + 1) * D) for h in heads])
    g2, _ = _g_table(heads)
    return {
        "xT": np.ascontiguousarray(x[b].T),
        "wq": np.ascontiguousarray(Wq[:, cols]),
        "wk": np.ascontiguousarray(Wk[:, cols]),
        "wv": np.ascontiguousarray(Wv[:, cols]),
        "wo": np.ascontiguousarray(Wo),
        "bqt": np.ascontiguousarray(bq[cols].reshape(NM, 128).T),
        "bkt": np.ascontiguousarray(bk[cols].reshape(NM, 128).T),
        "bv": np.ascontiguousarray(bv[cols].reshape(1, EC)),
        "gt": g2,
        "mk": _masks(),
    }


def kernel(x, Wq, bq, Wk, bk, Wv, bv, Wo, bo):
    global LAST_RESULT
    x = np.asarray(x, dtype=np.float32)
    Wq = np.asarray(Wq, dtype=np.float32)
    bq = np.asarray(bq, dtype=np.float32)
    Wk = np.asarray(Wk, dtype=np.float32)
    bk = np.asarray(bk, dtype=np.float32)
    Wv = np.asarray(Wv, dtype=np.float32)
    bv = np.asarray(bv, dtype=np.float32)
    Wo = np.asarray(Wo, dtype=np.float32)
    bo = np.asarray(bo, dtype=np.float32)

    nc = _build()
    in_maps = [
        _core_inputs(x, Wq, bq, Wk, bk, Wv, bv, Wo, c // 2, c % 2)
        for c in range(8)
    ]
    res = bass_utils.run_bass_kernel_spmd(nc, in_maps, core_ids=list(range(8)))
    LAST_RESULT = res
    out = np.zeros((B, S, E), dtype=np.float32)
    for c in range(8):
        b, hg = c // 2, c % 2
        heads = HEADS_A if hg == 0 else HEADS_B
        ys = res.results[c]["y"]            # [HL, NC, E]
        for l, h in enumerate(heads):
            r0 = (S * h - PHI[l]) // 12
            cnt = (S - 1 + PHI[l]) // 12 + 1
            out[b, r0:r0 + cnt] += ys[l, :cnt]
    out += bo
    return out


# revision 27
# speedup vs baseline: 1.0695x; 1.0695x over previous
"""Causal self-attention with ALiBi for Trainium2 (8 NeuronCores).

Sharding: 8 cores = 4 batch x 2 head-groups. Head-groups are interleaved
(even/odd ranks of ALiBi slope) so the exact-zero tile skipping (see below)
is load-balanced and the SPMD program is identical on every core.

Math: reference computes softmax(q.k/8 + slope*(q-k), causal) @ v.
We factor exp(q.k/8 + slope*(q-k) - slope*q - C) = exp(q.k/8) * g[k],
g[k] = exp(-slope*k - C).  g is folded into V (and into the appended
"ones" column that accumulates the softmax denominator), so the ACT
exp is a pure exp(x/8) with no per-row bias and the whole attention
runs in the transposed [k-partition, q-free] layout:

  S_T[k,q]  = k . q            (PE, fp32r, 2 heads packed via row tiling)
  P_T[k,q]  = exp(S_T/8)       (ACT, batched over 2 k-tiles)
  acc[d,q] += (v*g | g)^T P_T  (PE, M=65: 64 dims + denominator row)
  o_T[d,q]  = acc[d,q] / acc[64,q]
  y[q,e]    = o_T^T @ Wo       (PE, natural output layout)

Tiles where g underflows to exactly 0.0 in fp32 contribute exact zeros
and are skipped (host-verified per head; program uses the max keep over
the two head-groups so all 8 cores share one NEFF).
"""

import os
import sys

sys.path.insert(0, "/opt/trn_rl_repo")

import numpy as np

import concourse.bass as bass
import concourse.mybir as mybir
import concourse.tile as tile
from concourse import bacc
from concourse import bass_utils

F32 = mybir.dt.float32
F32R = mybir.dt.float32r
EXP = mybir.ActivationFunctionType.Exp

ALIBI_SLOPES = [
    0.6299605249, 0.396850263, 0.25, 0.1574901312,
    0.0992125657, 0.0625, 0.0393725328, 0.0248031414,
    0.015625, 0.0098431332, 0.0062007854, 0.00390625,
]
B, S, E, H, D = 4, 2048, 768, 12, 64
HL = 6                 # local heads per core
EC = HL * D            # 384 local embed
NJ = 4                 # q chunks of 512
QW = 512               # q chunk width
NK = 16                # k tiles of 128
NEC = 6                # E contraction tiles (768/128)
NM = 3                 # local e' tiles (384/128)
CSHIFT = 4.0           # softmax shift constant C
NEG = -1.0e30

# Slot l on group-A cores holds HEADS_A[l], on group-B cores HEADS_B[l].
# Pairing constraint: (2048*h) % 12 must match per slot (PHI) so the
# output-scramble program is identical on all cores; within that, heads
# are paired to balance the ALiBi-decay tile skipping.
HEADS_A = [0, 6, 2, 8, 4, 10]
HEADS_B = [3, 9, 5, 11, 1, 7]
PHI = [0, 0, 4, 4, 8, 8]
NC = 172               # z columns per slot (= ceil((2047+8)/12)+1)


def _g_table(heads):
    """[128, 96] fp32: g2[p, 6*jk + l] = exp(-slope_l*(128*jk+p) - C)."""
    k = np.arange(S, dtype=np.float64)
    sl = np.array([ALIBI_SLOPES[h] for h in heads], dtype=np.float64)
    g = np.exp(-np.outer(sl, k) - CSHIFT).astype(np.float32)  # [6, 2048]
    a = g.reshape(HL, NK, 128).transpose(2, 1, 0)             # [128, 16, 6]
    return np.ascontiguousarray(a.reshape(128, NK * HL)), a


def _keep_per_slot():
    """Per local-head-slot number of k-tiles with any nonzero g, maxed
    over the two head groups (so one program serves all cores)."""
    _, ga = _g_table(HEADS_A)
    _, gb = _g_table(HEADS_B)
    keep = []
    for l in range(HL):
        kp = 1
        for jk in range(NK):
            if (ga[:, jk, l] > 0).any() or (gb[:, jk, l] > 0).any():
                kp = jk + 1
        keep.append(kp)
    return keep


KEEP = _keep_per_slot()


def _masks():
    """[128, 4*512] additive causal masks for the 4 diagonal sub-positions.
    m[p, jd*512 + c] = 0 if 128*jd + p <= c else -1e30."""
    p = np.arange(128)[:, None]
    out = np.zeros((128, 4, QW), dtype=np.float32)
    for jd in range(4):
        c = np.arange(QW)[None, :]
        out[:, jd, :] = np.where(128 * jd + p <= c, 0.0, NEG)
    return np.ascontiguousarray(out.reshape(128, 4 * QW))


def _emit(tc, io):
    nc = tc.nc
    xT, wq, wk, wv, wo, bqt, bkt, bv, gt, mk, y = (
        io["xT"], io["wq"], io["wk"], io["wv"], io["wo"], io["bqt"],
        io["bkt"], io["bv"], io["gt"], io["mk"], io["y"])

    import contextlib
    ctx = tc._emit_ctx = contextlib.ExitStack()

    # ---- early pools (allocated below xw so attention tiles never wait
    # on the projection phase's address space) ----
    consts = ctx.enter_context(tc.tile_pool(name="consts", bufs=1))
    big = ctx.enter_context(tc.tile_pool(name="big", bufs=1))

    # packed small constants: [bqt | bkt | g | bv] = 3 + 3 + 96 + 384 cols
    small = consts.tile([128, 486], F32, name="small")
    bqt_sb = small[:, 0:NM]
    bkt_sb = small[:, NM:2 * NM]
    g_sb = small[:, 6:6 + NK * HL]
    bv_sb = small[:, 102:102 + EC]
    mk_sb = consts.tile([128, 4, QW], F32, name="mk_sb")

    qT = big.tile([128, NM, S], F32R, name="qT")
    kT = big.tile([128, NM, S], F32R, name="kT")
    v_all = big.tile([128, NK, HL * 65], F32R, name="v_all")

    nc.gpsimd.dma_start(out=g_sb, in_=gt)
    nc.gpsimd.dma_start(out=bqt_sb, in_=bqt)
    nc.gpsimd.dma_start(out=bkt_sb, in_=bkt)
    nc.gpsimd.dma_start(out=mk_sb[:, :, :],
                        in_=mk.rearrange("p (a b) -> p a b", a=4))
    nc.gpsimd.dma_start(
        out=bv_sb,
        in_=bass.AP(tensor=bv.tensor, offset=0, ap=[[0, 128], [1, EC]]))

    # ---- psum pools ----
    pp = ctx.enter_context(tc.tile_pool(name="pp", bufs=2, space="PSUM"))
    sc = ctx.enter_context(tc.tile_pool(name="sc", bufs=2, space="PSUM"))
    accp = ctx.enter_context(tc.tile_pool(name="accp", bufs=2, space="PSUM"))

    pt = ctx.enter_context(tc.tile_pool(name="pt", bufs=3))
    acs = ctx.enter_context(tc.tile_pool(name="acs", bufs=8))

    def proj_qk(xT_sb, w_sb, b_sb, dst, m):
        for j in range(NJ):
            ps = pp.tile([128, QW], F32, name="ps_qk", tag="pp")
            for c in range(NEC):
                nc.tensor.matmul(
                    ps,
                    lhsT=w_sb[:, c, 128 * m:128 * (m + 1)],
                    rhs=xT_sb[:, c, QW * j:QW * (j + 1)],
                    start=(c == 0), stop=(c == NEC - 1))
            nc.vector.tensor_scalar_add(
                out=dst[:, m, QW * j:QW * (j + 1)], in0=ps,
                scalar1=b_sb[:, m:m + 1])

    # ---- projections; qk m=0 first so attention exp starts early ----
    with tc.tile_pool(name="xw", bufs=1) as xw:
        xT_sb = xw.tile([128, NEC, S], F32R, name="xT_sb")
        wq_sb = xw.tile([128, NEC, EC], F32R, name="wq_sb")
        wk_sb = xw.tile([128, NEC, EC], F32R, name="wk_sb")
        wv_sb = xw.tile([128, NEC, EC], F32R, name="wv_sb")
        for c in range(NEC):
            eng = nc.sync if c % 2 == 0 else nc.scalar
            eng.dma_start(out=xT_sb[:, c, :], in_=xT[128 * c:128 * (c + 1), :])
        for c in range(NEC):
            nc.gpsimd.dma_start(out=wq_sb[:, c, :], in_=wq[128 * c:128 * (c + 1), :])
            nc.gpsimd.dma_start(out=wk_sb[:, c, :], in_=wk[128 * c:128 * (c + 1), :])
            nc.gpsimd.dma_start(out=wv_sb[:, c, :], in_=wv[128 * c:128 * (c + 1), :])

        def attn_pair(t):
            """QK -> mask -> exp -> PV -> psum->sbuf copy for slot pair t."""
            for j in range(NJ):
                lims = [min(4 * j + 4, KEEP[2 * t + hh]) for hh in range(2)]
                ngroups = (max(lims) + 1) // 2
                accs = []
                for hh in range(2):
                    acc = accp.tile([65, QW], F32, name="acc", tag="acc")
                    accs.append(acc)
                for g in range(ngroups):
                    sco = [None, None]
                    jks = [None, None]
                    for hh in range(2):
                        jj = [k for k in (2 * g, 2 * g + 1) if k < lims[hh]]
                        jks[hh] = jj
                        if not jj:
                            continue
                        s_ps = sc.tile([128, len(jj), QW], F32, name="s_ps",
                                       tag="sc")
                        sco[hh] = s_ps
                        for i, jk in enumerate(jj):
                            nc.tensor.matmul(
                                s_ps[:, i, :],
                                lhsT=kT[64 * hh:64 * hh + 64, t,
                                        128 * jk:128 * (jk + 1)],
                                rhs=qT[64 * hh:64 * hh + 64, t,
                                       QW * j:QW * (j + 1)],
                                start=True, stop=True,
                                tile_position=(64 * hh, 0))
                    for hh in range(2):
                        if sco[hh] is None:
                            continue
                        for i, jk in enumerate(jks[hh]):
                            if jk >= 4 * j:
                                jd = jk - 4 * j
                                nc.vector.tensor_add(
                                    sco[hh][:, i, 128 * jd:128 * (jd + 1)],
                                    sco[hh][:, i, 128 * jd:128 * (jd + 1)],
                                    mk_sb[:, jd, 128 * jd:128 * (jd + 1)])
                        p_sb = pt.tile([128, len(jks[hh]), QW], F32R,
                                       name="p_sb", tag="pt")
                        nc.scalar.activation(out=p_sb, in_=sco[hh], func=EXP,
                                             bias=0.0, scale=0.125)
                        for i, jk in enumerate(jks[hh]):
                            jd = jk - 4 * j
                            if jd >= 1:
                                nc.gpsimd.memset(
                                    p_sb[:, i, 0:128 * jd].bitcast(F32), 0.0)
                        l = 2 * t + hh
                        for i, jk in enumerate(jks[hh]):
                            nc.tensor.matmul(
                                accs[hh],
                                lhsT=v_all[:, jk, 65 * l:65 * (l + 1)],
                                rhs=p_sb[:, i, :],
                                start=(jk == 0), stop=(jk == lims[hh] - 1))
                # copy acc out of PSUM immediately (frees the bank)
                for hh in range(2):
                    ac = acs.tile([65, QW], F32, name="ac", tag="ac")
                    nc.vector.tensor_copy(out=ac, in_=accs[hh])
                    ac_tiles[(t, j, hh)] = ac

        proj_qk(xT_sb, wq_sb, bqt_sb, qT, 0)
        proj_qk(xT_sb, wk_sb, bkt_sb, kT, 0)
        ac_tiles = {}
        attn_pair(0)

        for jk in range(NK):
            psv = pp.tile([128, EC], F32, name="psv", tag="pp")
            for c in range(NEC):
                nc.tensor.matmul(
                    psv,
                    lhsT=xT_sb[:, c, 128 * jk:128 * (jk + 1)],
                    rhs=wv_sb[:, c, :],
                    start=(c == 0), stop=(c == NEC - 1))
            vst = xw.tile([128, EC], F32, name="vst", tag="vst", bufs=1)
            nc.vector.tensor_add(vst, psv, bv_sb)
            for l in range(HL):
                nc.vector.tensor_scalar_mul(
                    out=v_all[:, jk, 65 * l:65 * l + 64],
                    in0=vst[:, 64 * l:64 * (l + 1)],
                    scalar1=g_sb[:, HL * jk + l:HL * jk + l + 1])
            # denominator column = g
            nc.vector.tensor_copy(
                out=v_all[:, jk, :].rearrange("p (l c) -> p l c", c=65)[:, :, 64:65],
                in_=g_sb[:, HL * jk:HL * (jk + 1)].rearrange(
                    "p (l o) -> p l o", o=1))

        proj_qk(xT_sb, wq_sb, bqt_sb, qT, 1)
        proj_qk(xT_sb, wk_sb, bkt_sb, kT, 1)
        proj_qk(xT_sb, wq_sb, bqt_sb, qT, 2)
        proj_qk(xT_sb, wk_sb, bkt_sb, kT, 2)

    # ---- late pools (reuse the projection phase's address space) ----
    att = ctx.enter_context(tc.tile_pool(name="att", bufs=1))
    oT = att.tile([128, NM, S], F32, name="oT")
    z_all = att.tile([128, HL, 6, NC], F32R, name="z_all")
    wo_sb = att.tile([128, 6, E], F32R, name="wo_sb")
    for t in range(6):
        nc.scalar.dma_start(out=wo_sb[:, t, :], in_=wo[128 * t:128 * (t + 1), :])
    nc.vector.memset(z_all.bitcast(F32), 0.0)
    ypool = ctx.enter_context(tc.tile_pool(name="ypool", bufs=2))
    nrm = ctx.enter_context(tc.tile_pool(name="nrm", bufs=2))

    def tail_pair(t):
        """normalize + scramble + out-projection for slot pair t."""
        for j in range(NJ):
            for hh in range(2):
                ac = ac_tiles[(t, j, hh)]
                rr = nrm.tile([1, QW], F32, name="rr", tag="rr")
                nc.vector.reciprocal(out=rr, in_=ac[64:65, :])
                bc = nrm.tile([64, QW], F32, name="bc", tag="bc")
                nc.gpsimd.partition_broadcast(bc, rr)
                nc.vector.tensor_mul(
                    oT[64 * hh:64 * hh + 64, t, QW * j:QW * (j + 1)],
                    ac[0:64, :], bc)
        for hh in range(2):
            l = 2 * t + hh
            phi = PHI[l]
            for jb in range(12):
                s0 = (jb - phi) % 12
                cnt = (S - 1 - s0) // 12 + 1
                c0 = 1 if jb < phi else 0
                nc.vector.tensor_copy(
                    out=z_all[64 * (jb % 2):64 * (jb % 2) + 64, l, jb // 2,
                              c0:c0 + cnt],
                    in_=oT[64 * hh:64 * hh + 64, t,
                           s0:s0 + 12 * (cnt - 1) + 1:12])
        for hh in range(2):
            l = 2 * t + hh
            for cc, (r0, r1) in enumerate(((0, 128), (128, NC))):
                yst = ypool.tile([128, E], F32, name="yst", tag="y")
                for n in range(2):
                    psy = pp.tile([128, EC], F32, name="psy", tag="pp")
                    for m in range(NM * 2):
                        nc.tensor.matmul(
                            psy[0:r1 - r0, :],
                            lhsT=z_all[:, l, m, r0:r1],
                            rhs=wo_sb[:, m, EC * n:EC * (n + 1)],
                            start=(m == 0), stop=(m == NM * 2 - 1))
                    nc.vector.tensor_copy(out=yst[0:r1 - r0, EC * n:EC * (n + 1)],
                                          in_=psy[0:r1 - r0, :])
                nc.sync.dma_start(out=y[l, r0:r1, :], in_=yst[0:r1 - r0, :])

    attn_pair(1)
    tail_pair(0)
    attn_pair(2)
    tail_pair(1)
    tail_pair(2)

    ctx.close()


_BUILT = None
LAST_RESULT = None


def _build():
    global _BUILT
    if _BUILT is not None:
        return _BUILT
    nc = bacc.Bacc("TRN2", target_bir_lowering=False, debug=False,
                   enable_asserts=False, num_devices=8)
    io = {
        "xT": nc.dram_tensor("xT", [E, S], F32R, kind="ExternalInput").ap(),
        "wq": nc.dram_tensor("wq", [E, EC], F32R, kind="ExternalInput").ap(),
        "wk": nc.dram_tensor("wk", [E, EC], F32R, kind="ExternalInput").ap(),
        "wv": nc.dram_tensor("wv", [E, EC], F32R, kind="ExternalInput").ap(),
        "wo": nc.dram_tensor("wo", [E, E], F32R, kind="ExternalInput").ap(),
        "bqt": nc.dram_tensor("bqt", [128, NM], F32, kind="ExternalInput").ap(),
        "bkt": nc.dram_tensor("bkt", [128, NM], F32, kind="ExternalInput").ap(),
        "bv": nc.dram_tensor("bv", [1, EC], F32, kind="ExternalInput").ap(),
        "gt": nc.dram_tensor("gt", [128, NK * HL], F32,
                             kind="ExternalInput").ap(),
        "mk": nc.dram_tensor("mk", [128, 4 * QW], F32,
                             kind="ExternalInput").ap(),
        "y": nc.dram_tensor("y", [HL, NC, E], F32, kind="ExternalOutput").ap(),
    }
    with tile.TileContext(nc) as tc:
        _emit(tc, io)
    nc.compile()
    _BUILT = nc
    return nc


def _core_inputs(x, Wq, bq, Wk, bk, Wv, bv, Wo, b, hg):
    heads = HEADS_A if hg == 0 else HEADS_B
    cols = np.concatenate([np.arange(h * D, (h + 1) * D) for h in heads])
    g2, _ = _g_table(heads)
    return {
        "xT": np.ascontiguousarray(x[b].T),
        "wq": np.ascontiguousarray(Wq[:, cols]),
        "wk": np.ascontiguousarray(Wk[:, cols]),
        "wv": np.ascontiguousarray(Wv[:, cols]),
        "wo": np.ascontiguousarray(Wo),
        "bqt": np.ascontiguousarray(bq[cols].reshape(NM, 128).T),
        "bkt": np.ascontiguousarray(bk[cols].reshape(NM, 128).T),
        "bv": np.ascontiguousarray(bv[cols].reshape(1, EC)),
        "gt": g2,
        "mk": _masks(),
    }


def kernel(x, Wq, bq, Wk, bk, Wv, bv, Wo, bo):
    global LAST_RESULT
    x = np.asarray(x, dtype=np.float32)
    Wq = np.asarray(Wq, dtype=np.float32)
    bq = np.asarray(bq, dtype=np.float32)
    Wk = np.asarray(Wk, dtype=np.float32)
    bk = np.asarray(bk, dtype=np.float32)
    Wv = np.asarray(Wv, dtype=np.float32)
    bv = np.asarray(bv, dtype=np.float32)
    Wo = np.asarray(Wo, dtype=np.float32)
    bo = np.asarray(bo, dtype=np.float32)

    nc = _build()
    in_maps = [
        _core_inputs(x, Wq, bq, Wk, bk, Wv, bv, Wo, c // 2, c % 2)
        for c in range(8)
    ]
    res = bass_utils.run_bass_kernel_spmd(nc, in_maps, core_ids=list(range(8)))
    LAST_RESULT = res
    out = np.zeros((B, S, E), dtype=np.float32)
    for c in range(8):
        b, hg = c // 2, c % 2
        heads = HEADS_A if hg == 0 else HEADS_B
        ys = res.results[c]["y"]            # [HL, NC, E]
        for l, h in enumerate(heads):
            r0 = (S * h - PHI[l]) // 12
            cnt = (S - 1 + PHI[l]) // 12 + 1
            out[b, r0:r0 + cnt] += ys[l, :cnt]
    out += bo
    return out
